# revision 1
# baseline (speedup 1.0000x reference)
"""Trainium2 Bass kernel for a pre-LN transformer block (B=4096, T=64, C=256, H=4, D=64).

Data-parallel over 8 NeuronCores: batch split 512 seqs/core, weights replicated.
Fully fused, software-pipelined over 8-sequence chunks (512 tokens):
  S1: load x, LN1, transpose, QKV
  S2: causal attention (no max-sub; scores are small), proj + residual, LN2
  S3: MLP(relu) + residual, store
Stages are emitted with a 1-chunk skew (S1(k), S2(k-1), S3(k-2)) so each
engine's instruction stream interleaves independent chunks.
Matmuls in bf16 (fp32 PSUM accum); residual stream kept in fp32.
Attention computes S^T[s,t] per seq-pair so the softmax denominator is a
ones-matmul and normalization is a rank-1 broadcast matmul; PE row-tile packed
matmuls write to per-row-tile PSUM banks (same-bank concurrent row-tile
writes fault the device).
"""
import sys, os

sys.path.insert(0, "/opt/trn_rl_repo")

import numpy as np
import ml_dtypes

import concourse.bass as bass
import concourse.tile as tile
from concourse import bacc, mybir
from concourse.bass_utils import run_bass_kernel_spmd

# All ACT functions used here (Exp, Ln, Copy, Relu, Identity) live in the
# 'natural_log_exp_and_others' table set, but bacc's table chooser picks a
# canonical set per function and thrashes between natural_log and
# exp_and_others every chunk (~2.7us per ACT table swap).  Blank out every
# other set (order preserved -> act_func_set_ids stay valid) so the chooser
# must use the combined set; the load then hoists to one per kernel.
_orig_get_tables = bacc.get_activation_tables


def _combined_tables_only(arch):
    tabs = _orig_get_tables(arch)
    return {k: (v if k == "natural_log_exp_and_others" else set())
            for k, v in tabs.items()}


bacc.get_activation_tables = _combined_tables_only

F32 = mybir.dt.float32
BF16 = mybir.dt.bfloat16
AF = mybir.ActivationFunctionType
ALU = mybir.AluOpType

N_CORES = 8
B, T, C, H, D = 4096, 64, 256, 4, 64
BC = B // N_CORES            # 512 seqs per core
CHUNK_SEQ = 8                # sequences per chunk
TOK = CHUNK_SEQ * T          # 512 tokens per chunk
NT = TOK // 128              # 4 token-tiles per chunk
N_CHUNKS = BC // CHUNK_SEQ   # 64
EPS = 1e-6

_COMPILED = {}

BUF2 = int(os.environ.get("BUF2", "2"))    # intra-stage tiles
EP_BUFS = int(os.environ.get("EP_BUFS", "2"))   # attention e/p/pn tiles
GP_MASK = os.environ.get("GP_MASK", "0") == "1"     # mask-mul on gpsimd
GP_LN = os.environ.get("GP_LN", "0") == "1"         # LN applies on gpsimd
V_ACT = os.environ.get("V_ACT", "0") == "1"         # v drain on scalar engine
RELU_DVE_MOD = int(os.environ.get("RELU_DVE_MOD", "4"))  # f%mod==0 -> DVE
LN_V2 = os.environ.get("LN_V2", "0") == "1"
SKEW4 = os.environ.get("SKEW4", "1") == "1"
SKEW5 = os.environ.get("SKEW5", "0") == "1"
BUF3X = int(os.environ.get("BUF3X", "4"))  # x tile (longest lifetime)
BUF3 = int(os.environ.get("BUF3", "3"))    # stage-crossing tiles
SMALL_BUFS = int(os.environ.get("SMALL_BUFS", "3"))
PS_A = int(os.environ.get("PS_A", "2"))
PS_B = int(os.environ.get("PS_B", "3"))
PS_C = int(os.environ.get("PS_C", "3"))


def _build(n_chunks, stage="full"):
    nc = bacc.Bacc("TRN2", target_bir_lowering=False, debug=False,
                   enable_asserts=False, num_devices=N_CORES)

    ntok = n_chunks * TOK
    x_d = nc.dram_tensor("x", [ntok, C], F32, kind="ExternalInput")
    out_d = nc.dram_tensor("out", [ntok, C], F32, kind="ExternalOutput")
    wq_d = nc.dram_tensor("wq", [128, 512], BF16, kind="ExternalInput")
    wk_d = nc.dram_tensor("wk", [128, 512], BF16, kind="ExternalInput")
    wv_d = nc.dram_tensor("wv", [128, 512], BF16, kind="ExternalInput")
    wp_d = nc.dram_tensor("wp", [128, 512], BF16, kind="ExternalInput")
    w1_d = nc.dram_tensor("w1", [128, 2048], BF16, kind="ExternalInput")
    w2_d = nc.dram_tensor("w2", [128, 2048], BF16, kind="ExternalInput")
    msk_d = nc.dram_tensor("msk", [128, 512], BF16, kind="ExternalInput")
    idn_d = nc.dram_tensor("idn", [128, 128], BF16, kind="ExternalInput")
    onc_d = nc.dram_tensor("onc", [128, 1], BF16, kind="ExternalInput")
    onr_d = nc.dram_tensor("onr", [1, 128], BF16, kind="ExternalInput")

    with tile.TileContext(nc) as tc, nc.allow_low_precision("bf16 block kernel"):
        with tc.tile_pool(name="consts", bufs=1) as cp, \
             tc.tile_pool(name="acts", bufs=BUF2) as ap, \
             tc.tile_pool(name="small", bufs=SMALL_BUFS) as sp, \
             tc.tile_pool(name="psum", bufs=1, space="PSUM") as psp:

            def cload(dram, shape, dt=BF16):
                t = cp.tile(shape, dt, tag=dram.name + "_c", name=dram.name + "_c")
                nc.sync.dma_start(t[:], dram.ap())
                return t

            wq = cload(wq_d, [128, 512])
            wk = cload(wk_d, [128, 512])
            wv = cload(wv_d, [128, 512])
            wp = cload(wp_d, [128, 512])
            w1 = cload(w1_d, [128, 2048])
            w2 = cload(w2_d, [128, 2048])
            msk = cload(msk_d, [128, 512])
            idn = cload(idn_d, [128, 128])
            onc = cload(onc_d, [128, 1])
            onr = cload(onr_d, [1, 128])
            eps = cp.tile([128, 1], F32, name="eps")
            nc.vector.memset(eps[:], EPS)

            x_r = x_d.ap().rearrange("(k n p) c -> k p n c", p=128, n=NT)
            out_r = out_d.ap().rearrange("(k n p) c -> k p n c", p=128, n=NT)

            def layernorm(src_sb, dst_bf16, tag):
                """src [128, NT*256] fp32 -> dst bf16 normalized (no affine)."""
                src3 = src_sb.rearrange("p (n c) -> p n c", n=NT)
                rstd = sp.tile([128, NT], F32, tag=tag + "_rs", name=tag + "_rs")
                nmsr = sp.tile([128, NT], F32, tag=tag + "_nm", name=tag + "_nm")
                lnv = sp.tile([128, NT], F32, tag=tag + "_sd", name=tag + "_sd")
                varr = sp.tile([128, NT], F32, tag=tag + "_va", name=tag + "_va")
                sums = sp.tile([128, NT], F32, tag=tag + "_su", name=tag + "_su")
                if LN_V2:
                    # one free-axis reduce for sums; ACT Square w/ accum_out
                    # for sum-of-squares (scratch lands in dst, overwritten
                    # by the apply below)
                    nc.vector.reduce_sum(sums[:], src3, axis=mybir.AxisListType.X)
                    sqs = sp.tile([128, NT], F32, tag=tag + "_sq", name=tag + "_sq")
                    for n in range(NT):
                        nc.scalar.activation(
                            dst_bf16[:, n * 256:(n + 1) * 256],
                            src_sb[:, n * 256:(n + 1) * 256],
                            AF.Square, accum_out=sqs[:, n:n + 1])
                    msq = sp.tile([128, NT], F32, tag=tag + "_mq", name=tag + "_mq")
                    nc.vector.scalar_tensor_tensor(
                        msq[:], sums[:], 1.0 / (C * C), sums[:],
                        op0=ALU.mult, op1=ALU.mult)
                    nc.vector.scalar_tensor_tensor(
                        varr[:], sqs[:], 1.0 / C, msq[:],
                        op0=ALU.mult, op1=ALU.subtract)
                    var_ap, mean_ap, mean_scale = varr[:], sums[:], -1.0 / C
                else:
                    st = sp.tile([128, NT, 6], F32, tag=tag + "_st", name=tag + "_st")
                    mv = sp.tile([128, NT, 2], F32, tag=tag + "_mv", name=tag + "_mv")
                    for n in range(NT):
                        nc.vector.bn_stats(st[:, n, :], src3[:, n, :])
                        nc.vector.bn_aggr(mv[:, n, :], st[:, n, :])
                    var_ap, mean_ap, mean_scale = mv[:, :, 1], mv[:, :, 0], -1.0
                # rstd = (var+eps)^-0.5 = exp(-0.5*ln(var+eps)); Ln+Exp share
                # one ACT table set (sqrt would force a set swap every chunk)
                nc.scalar.activation(lnv[:], var_ap, AF.Ln, bias=eps[:])
                nc.scalar.activation(rstd[:], lnv[:], AF.Exp, scale=-0.5)
                nc.vector.scalar_tensor_tensor(
                    nmsr[:], mean_ap, mean_scale, rstd[:],
                    op0=ALU.mult, op1=ALU.mult)
                eng_ts = nc.gpsimd if GP_LN else nc.vector
                for n in range(NT):
                    eng_ts.tensor_scalar(
                        dst_bf16[:, n * 256:(n + 1) * 256],
                        src_sb[:, n * 256:(n + 1) * 256],
                        rstd[:, n:n + 1], nmsr[:, n:n + 1],
                        op0=ALU.mult, op1=ALU.add)

            def transpose_1024(src_bf16, tag, bufs):
                """src [128 tok, 1024] -> [128 c, 2, 512 tok] bf16."""
                dst = ap.tile([128, 2, TOK], BF16, tag=tag, name=tag, bufs=bufs)
                for ch in range(2):
                    tp = psp.tile([128, TOK], BF16, tag="A", bufs=PS_A, name="tp")
                    for n in range(NT):
                        nc.tensor.transpose(
                            tp[:, n * 128:(n + 1) * 128],
                            src_bf16[:, n * 256 + ch * 128: n * 256 + ch * 128 + 128],
                            idn[:])
                    nc.scalar.copy(dst[:, ch, :], tp[:])
                return dst

            def stage1a(k):
                x_sb = ap.tile([128, NT * 256], F32, tag="x", name="x", bufs=BUF3X)
                nc.sync.dma_start(
                    x_sb[:].rearrange("p (n c) -> p n c", n=NT), x_r[k])
                h_sb = ap.tile([128, NT * 256], BF16, tag="h", name="h")
                layernorm(x_sb[:], h_sb[:], "ln1")
                hT = transpose_1024(h_sb[:], "hT", BUF3 if SKEW5 else BUF2)
                return dict(x=x_sb, hT=hT)

            def stage1b(k, s):
                hT = s["hT"]
                qT_sb = ap.tile([128, 2, TOK], BF16, tag="qT", name="qT", bufs=BUF3)
                kT_sb = ap.tile([128, 2, TOK], BF16, tag="kT", name="kT", bufs=BUF3)
                for ph in range(2):
                    qp = psp.tile([128, TOK], F32, tag="A", bufs=PS_A, name="qp")
                    kp = psp.tile([128, TOK], F32, tag="A", bufs=PS_A, name="kp")
                    for ksl in range(2):
                        o = ph * 256 + ksl * 128
                        nc.tensor.matmul(qp[:], wq[:, o:o + 128], hT[:, ksl, :],
                                         start=(ksl == 0), stop=(ksl == 1))
                        nc.tensor.matmul(kp[:], wk[:, o:o + 128], hT[:, ksl, :],
                                         start=(ksl == 0), stop=(ksl == 1))
                    nc.scalar.copy(qT_sb[:, ph, :], qp[:])
                    nc.scalar.copy(kT_sb[:, ph, :], kp[:])
                v_sb = ap.tile([128, NT * 256], BF16, tag="v", name="v", bufs=BUF3)
                for m in range(0, NT, 2):
                    vp = psp.tile([128, 512], F32, tag="A", bufs=PS_A, name="vp")
                    for j in range(2):
                        for ksl in range(2):
                            nc.tensor.matmul(
                                vp[:, j * 256:(j + 1) * 256],
                                hT[:, ksl, (m + j) * 128:(m + j + 1) * 128],
                                wv[:, ksl * 256:(ksl + 1) * 256],
                                start=(ksl == 0), stop=(ksl == 1))
                    if V_ACT:
                        nc.scalar.copy(v_sb[:, m * 256:(m + 2) * 256], vp[:])
                    else:
                        nc.vector.tensor_copy(v_sb[:, m * 256:(m + 2) * 256], vp[:])
                return dict(qT=qT_sb, kT=kT_sb, v=v_sb)

            def stage2(k, s):
                x_sb, qT_sb, kT_sb, v_sb = s["x"], s["qT"], s["kT"], s["v"]
                attT_sb = ap.tile([128, 2, TOK], BF16, tag="attT", name="attT",
                  bufs=BUF3 if SKEW4 else BUF2)
                for q in range(2):          # seq-quad; phase-major over ph
                    s_ps, e_sb, p_sb, rcp, d4, pn_sb, at_ps = ({} for _ in range(7))
                    for ph in range(2):
                        s_ps[ph] = [psp.tile([128, 256], F32, tag="B", bufs=PS_B,
                                             name=f"s{hh}") for hh in range(2)]
                        for r in range(2):
                            for hh in range(2):
                                tcol = (4 * q + 2 * r) * 64
                                nc.tensor.matmul(
                                    s_ps[ph][hh][:, r * 128:(r + 1) * 128],
                                    kT_sb[hh * 64:hh * 64 + 64, ph, tcol:tcol + 128],
                                    qT_sb[hh * 64:hh * 64 + 64, ph, tcol:tcol + 128],
                                    start=True, stop=True,
                                    tile_position=(hh * 64, 0))
                    for ph in range(2):
                        e_sb[ph] = ap.tile([128, 512], BF16, tag="e", name="e",
                                           bufs=EP_BUFS)
                        nc.scalar.activation(e_sb[ph][:, 0:256], s_ps[ph][0][:], AF.Exp)
                        nc.scalar.activation(e_sb[ph][:, 256:512], s_ps[ph][1][:], AF.Exp)
                    for ph in range(2):
                        p_sb[ph] = ap.tile([128, 512], BF16, tag="p", name="p",
                                           bufs=EP_BUFS)
                        (nc.gpsimd if GP_MASK else nc.vector).tensor_tensor(
                            p_sb[ph][:], e_sb[ph][:], msk[:], op=ALU.mult)
                    # sums live in row 0 of the d4 tile; recip reads it, then
                    # the broadcast matmul overwrites the whole tile (WAR).
                    for ph in range(2):
                        d4[ph] = psp.tile([128, 512], F32, tag="B", bufs=PS_B,
                                          name="d4")
                        nc.tensor.matmul(d4[ph][0:1, :], onc[:], p_sb[ph][:],
                                         start=True, stop=True)
                    for ph in range(2):
                        rcp[ph] = sp.tile([1, 512], BF16, tag="rcp", name="rcp")
                        nc.vector.reciprocal(rcp[ph][:], d4[ph][0:1, :])
                    for ph in range(2):
                        nc.tensor.matmul(d4[ph][:], onr[:], rcp[ph][:],
                                         start=True, stop=True)
                    for ph in range(2):
                        pn_sb[ph] = ap.tile([128, 512], BF16, tag="pn", name="pn",
                                            bufs=EP_BUFS)
                        nc.vector.tensor_tensor(pn_sb[ph][:], p_sb[ph][:], d4[ph][:],
                                                op=ALU.mult)
                    for ph in range(2):
                        at_ps[ph] = [psp.tile([128, 128], F32, tag="B", bufs=PS_B,
                                              name=f"at{i}") for i in range(2)]
                        for r in range(2):
                            for hh in range(2):
                                for i in range(2):
                                    sq = 4 * q + 2 * r + i
                                    vm = sq // 2
                                    h_abs = 2 * ph + hh
                                    nc.tensor.matmul(
                                        at_ps[ph][i][hh * 64:hh * 64 + 64,
                                                     r * 64:(r + 1) * 64],
                                        v_sb[i * 64:i * 64 + 64,
                                             vm * 256 + h_abs * 64: vm * 256 + h_abs * 64 + 64],
                                        pn_sb[ph][i * 64:i * 64 + 64,
                                                  hh * 256 + r * 128 + i * 64:
                                                  hh * 256 + r * 128 + i * 64 + 64],
                                        start=True, stop=True,
                                        tile_position=(i * 64, hh * 64))
                    for ph in range(2):
                        dst4 = attT_sb[:, ph, q * 256:(q + 1) * 256].rearrange(
                            "p (r i t) -> p r i t", r=2, i=2)
                        for i in range(2):
                            nc.scalar.copy(
                                dst4[:, :, i, :],
                                at_ps[ph][i][:].rearrange("p (r t) -> p r t", r=2))

                return dict(attT=attT_sb)

            def stage2b(k, s):
                x_sb, attT_sb = s["x"], s["attT"]
                x2_sb = ap.tile([128, NT * 256], F32, tag="x2", name="x2", bufs=BUF3)
                for n2 in range(0, NT, 2):
                    sa = psp.tile([128, 512], F32, tag="C", bufs=PS_C, name="sa")
                    for j in range(2):
                        for ph in range(2):
                            nc.tensor.matmul(
                                sa[:, j * 256:(j + 1) * 256],
                                attT_sb[:, ph, (n2 + j) * 128:(n2 + j + 1) * 128],
                                wp[:, ph * 256:(ph + 1) * 256],
                                start=(ph == 0), stop=(ph == 1))
                    nc.vector.tensor_tensor(
                        x2_sb[:, n2 * 256:(n2 + 2) * 256],
                        x_sb[:, n2 * 256:(n2 + 2) * 256], sa[:], op=ALU.add)
                h2_sb = ap.tile([128, NT * 256], BF16, tag="h2", name="h2")
                layernorm(x2_sb[:], h2_sb[:], "ln2")
                h2T = transpose_1024(h2_sb[:], "h2T", BUF3)
                return dict(x2=x2_sb, h2T=h2T)

            def stage3(k, s):
                x2_sb, h2T = s["x2"], s["h2T"]
                zr_sb = ap.tile([128, 8 * TOK], BF16, tag="zr", name="zr")
                for f in range(8):
                    zp = psp.tile([128, TOK], F32, tag="C", bufs=PS_C, name="zp")
                    for ksl in range(2):
                        nc.tensor.matmul(
                            zp[:],
                            w1[:, ksl * 1024 + f * 128: ksl * 1024 + (f + 1) * 128],
                            h2T[:, ksl, :],
                            start=(ksl == 0), stop=(ksl == 1))
                    if f % RELU_DVE_MOD == 0:
                        nc.vector.tensor_scalar_max(
                            zr_sb[:, f * TOK:(f + 1) * TOK], zp[:], 0.0)
                    else:
                        nc.scalar.activation(
                            zr_sb[:, f * TOK:(f + 1) * TOK], zp[:], AF.Relu)
                out_sb = ap.tile([128, NT * 256], F32, tag="o", name="o")
                for n2 in range(0, NT, 2):
                    yp = psp.tile([128, 512], F32, tag="C", bufs=PS_C, name="yp")
                    for j in range(2):
                        n = n2 + j
                        for f in range(8):
                            nc.tensor.matmul(
                                yp[:, j * 256:(j + 1) * 256],
                                zr_sb[:, f * TOK + n * 128: f * TOK + (n + 1) * 128],
                                w2[:, f * 256:(f + 1) * 256],
                                start=(f == 0), stop=(f == 7))
                    nc.vector.tensor_tensor(
                        out_sb[:, n2 * 256:(n2 + 2) * 256],
                        x2_sb[:, n2 * 256:(n2 + 2) * 256], yp[:], op=ALU.add)
                nc.sync.dma_start(
                    out_r[k], out_sb[:].rearrange("p (n c) -> p n c", n=NT))

            def emit_all():
                if SKEW5:
                    st = {}
                    for kk in range(n_chunks + 4):
                        if kk < n_chunks:
                            st[kk] = stage1a(kk)
                        if 0 <= kk - 1 < n_chunks:
                            st[kk - 1].update(stage1b(kk - 1, st[kk - 1]))
                        if 0 <= kk - 2 < n_chunks:
                            st[kk - 2].update(stage2(kk - 2, st[kk - 2]))
                        if 0 <= kk - 3 < n_chunks:
                            st[kk - 3].update(stage2b(kk - 3, st[kk - 3]))
                        if 0 <= kk - 4 < n_chunks:
                            stage3(kk - 4, st.pop(kk - 4))
                elif SKEW4:
                    st = {}
                    for kk in range(n_chunks + 3):
                        if kk < n_chunks:
                            st[kk] = stage1a(kk)
                            st[kk].update(stage1b(kk, st[kk]))
                        if 0 <= kk - 1 < n_chunks:
                            st[kk - 1].update(stage2(kk - 1, st[kk - 1]))
                        if 0 <= kk - 2 < n_chunks:
                            st[kk - 2].update(stage2b(kk - 2, st[kk - 2]))
                        if 0 <= kk - 3 < n_chunks:
                            stage3(kk - 3, st.pop(kk - 3))
                else:
                    st = {}
                    for kk in range(n_chunks + 2):
                        if kk < n_chunks:
                            st[kk] = stage1a(kk)
                            st[kk].update(stage1b(kk, st[kk]))
                        if 0 <= kk - 1 < n_chunks:
                            st[kk - 1].update(stage2(kk - 1, st[kk - 1]))
                            st[kk - 1].update(stage2b(kk - 1, st[kk - 1]))
                        if 0 <= kk - 2 < n_chunks:
                            stage3(kk - 2, st.pop(kk - 2))

            rep = int(os.environ.get("BENCH_REPEAT", "1"))
            if rep > 1:
                with tc.For_i(0, rep, 1):
                    emit_all()
            else:
                emit_all()

    nc.compile()
    return nc


def _prep_consts(ln1_g, Wq, Wk, Wv, Wproj, ln2_g, W1, W2):
    bf = ml_dtypes.bfloat16
    scale = 1.0 / np.sqrt(np.float32(D))
    Wq = (Wq * ln1_g[None, :, None] * scale).astype(np.float32)
    Wk = (Wk * ln1_g[None, :, None]).astype(np.float32)
    Wv = (Wv * ln1_g[None, :, None]).astype(np.float32)
    W1 = (W1 * ln2_g[:, None]).astype(np.float32)

    def pack_qk(W):  # [H,C,D] -> [128, 512]: col = ph*256 + ksl*128 + m
        out = np.zeros((128, 512), np.float32)
        for ph in range(2):
            m = np.concatenate([W[2 * ph], W[2 * ph + 1]], axis=1)  # [C, 128]
            for ksl in range(2):
                out[:, ph * 256 + ksl * 128: ph * 256 + (ksl + 1) * 128] = \
                    m[ksl * 128:(ksl + 1) * 128, :]
        return out.astype(bf)

    wv_p = np.zeros((128, 512), np.float32)
    Wv_f = np.transpose(Wv, (1, 0, 2)).reshape(C, H * D)
    for ksl in range(2):
        wv_p[:, ksl * 256:(ksl + 1) * 256] = Wv_f[ksl * 128:(ksl + 1) * 128, :]
    wp_p = np.zeros((128, 512), np.float32)
    for ph in range(2):
        wp_p[:, ph * 256:(ph + 1) * 256] = Wproj[ph * 128:(ph + 1) * 128, :]
    w1_p = np.zeros((128, 2048), np.float32)
    for ksl in range(2):
        for f in range(8):
            w1_p[:, ksl * 1024 + f * 128: ksl * 1024 + (f + 1) * 128] = \
                W1[ksl * 128:(ksl + 1) * 128, f * 128:(f + 1) * 128]
    w2_p = np.zeros((128, 2048), np.float32)
    for f in range(8):
        w2_p[:, f * 256:(f + 1) * 256] = W2[f * 128:(f + 1) * 128, :]

    tri = (np.arange(64)[:, None] <= np.arange(64)[None, :]).astype(np.float32)
    blk = np.zeros((128, 128), np.float32)
    blk[0:64, 0:64] = tri
    blk[64:128, 64:128] = tri
    msk = np.tile(blk, (1, 4))

    return {
        "wq": pack_qk(Wq), "wk": pack_qk(Wk),
        "wv": wv_p.astype(bf), "wp": wp_p.astype(bf),
        "w1": w1_p.astype(bf), "w2": w2_p.astype(bf),
        "msk": msk.astype(bf), "idn": np.eye(128, dtype=np.float32).astype(bf),
        "onc": np.ones((128, 1), np.float32).astype(bf),
        "onr": np.ones((1, 128), np.float32).astype(bf),
    }


def kernel(x, ln1_g, ln1_b, Wq, Wk, Wv, Wproj, bproj, ln2_g, ln2_b, W1, b1, W2, b2,
           n_chunks=N_CHUNKS, _results_only=False, trace=False):
    x = np.asarray(x, np.float32)
    for nm, b in (("ln1_b", ln1_b), ("bproj", bproj), ("ln2_b", ln2_b),
                  ("b1", b1), ("b2", b2)):
        if np.any(np.asarray(b) != 0):
            raise NotImplementedError(f"nonzero {nm} not supported")

    if n_chunks not in _COMPILED:
        _COMPILED[n_chunks] = _build(n_chunks)
    nc = _COMPILED[n_chunks]

    consts = _prep_consts(np.asarray(ln1_g, np.float32), np.asarray(Wq, np.float32),
                          np.asarray(Wk, np.float32), np.asarray(Wv, np.float32),
                          np.asarray(Wproj, np.float32), np.asarray(ln2_g, np.float32),
                          np.asarray(W1, np.float32), np.asarray(W2, np.float32))

    ntok = n_chunks * TOK
    xs = x.reshape(N_CORES, BC * T, C)[:, :ntok, :]
    in_maps = [dict(consts, x=np.ascontiguousarray(xs[i])) for i in range(N_CORES)]
    res = run_bass_kernel_spmd(nc, in_maps, list(range(N_CORES)), trace=trace)
    outs = np.stack([res.results[i]["out"] for i in range(N_CORES)])
    if _results_only:
        return outs, res
    assert ntok == BC * T
    return outs.reshape(B, T, C)

